# revision 1
# baseline (speedup 1.0000x reference)
"""Trainium2 Bass kernel for an FFM (field-aware factorization machine) forward pass.

Reference computation (all fp32):
    12 embedding matmuls over column slices of fv [32768, 2668], 15 pairwise
    dot-product cross terms, a linear layer and a sigmoid.

Every engine on TRN2 charges ~(free/moving size) cycles per instruction
regardless of partition rows, so the design minimizes instruction count per
512-batch sub-tile and balances the per-op costs across engines
(PE matmul ~213-300ns, DVE TSS 194ns / TT 327ns / STT 594ns, Pool STT
806ns, ACT ~770ns):

  * The 12 embeddings are packed as 64-row halves of 128-row weight blocks:
      A = [uu | ui]            tiles 0..7   (userid region)
      B = [ti | tu+mu]         tiles 7..20  (itemid region + movie@t20)
      D = [au+gu+ou | ai+gi+oi], E = [mi | au+gu], F = [au | au+ou]  (t20)
    with halves positioned so every cross product pairs operands at the
    SAME base partition (a hard SBUF tensor_tensor constraint).
    25 block matmuls + 3 reduce matmuls per sub on the PE.
  * Cross terms are 5 DVE tensor_tensor products on drained fp16 halves,
    using au*gu+au*ou+gu*ou = (au+gu)*(au+ou) - au*au; the -au*au side
    rides the linear chain via a host-derived fv row holding fv_2626^2
    with linear weight -||A_u||^2.
  * The linear term is split: ~10 K-tiles run on the PE as M=1 fp16
    matmuls into the logit PSUM (grouped after the blocks - interleaving
    1-row and 128-row outputs costs ~340ns of pipeline drain per switch),
    the rest on the DVE as scalar_tensor_tensor accumulate chains.
  * PSUM is drained to fp16 SBUF on the ACT engine; all DVE operands are
    then 2-byte + SBUF-only, enabling the DVE fast modes.
  * Sub n's products run one sub late and its reduce/sigmoid retire after
    sub n+1's block matmuls are queued (software pipelining), so the
    in-order PE/DVE queues never stall on the previous sub's tail.
  * fv is streamed as ONE contiguous [128, 21*1024] fp16 DMA per 1024-batch
    super-chunk (host pre-arranges the layout), split across two HWDGE
    rings - 2 descriptors instead of 21 per super.

Distribution: data-parallel over the batch dim - each of the 8 cores gets
4096 rows, cast to fp16 host-side (halves HBM traffic; rel err ~4e-3 vs
the 2e-2 gate).
"""

import os
import numpy as np
from contextlib import ExitStack

B, F, D = 32768, 2668, 64
NCORES = 8
BL = B // NCORES          # batch rows per core
NKT = 21                  # feature K-tiles of 128
FP = NKT * 128            # padded feature dim (2688)
SUPER = 1024              # batch columns per DMA chunk
NSUB = 512                # matmul moving-dim (one fp32 PSUM bank)
NSUPER = BL // SUPER
SW = NKT * SUPER          # super width in the packed fv layout

# w_pack column layout: A t0..7 | B t7..20 | D | E | F  (128 cols per tile)
A_TILES = tuple(range(0, 8))
B_TILES = tuple(range(7, 21))
AOFF = {t: i * 128 for i, t in enumerate(A_TILES)}
BOFF = {t: (8 + i) * 128 for i, t in enumerate(B_TILES)}
DOFF, EOFF, FOFF = 22 * 128, 23 * 128, 24 * 128
WF = 25 * 128

# linear-term K-tile split across engines (tunable; DVE pays ~660ns/tile as
# an STT, the PE ~205ns as an M=1 fp16 matmul into the logit PSUM, provided
# all M=1 matmuls are GROUPED so the PE pipeline reconfigures only once)
PE_LIN = int(os.environ.get("FFM_PE_LIN", "10"))
LIN_PE_TILES = tuple(t for t in range(NKT) if t % 2 == 1)[:PE_LIN] + \
    tuple(t for t in range(NKT) if t % 2 == 0)[:max(0, PE_LIN - 10)]


def _build_w_pack(inp):
    """Pack the block tables into one [128, WF] array laid out exactly as
    the SBUF weight tile wants it (partition k = row-in-K-tile)."""
    A_u, A_i = inp["age_user_w"], inp["age_item_w"]
    G_u, G_i = inp["gender_user_w"], inp["gender_item_w"]
    O_u, O_i = inp["occupation_user_w"], inp["occupation_item_w"]
    M_u, M_i = inp["movie_user_w"], inp["movie_item_w"]
    U_u, U_i = inp["userid_user_w"], inp["userid_item_w"]
    T_u, T_i = inp["itemid_user_w"], inp["itemid_item_w"]

    WA = np.zeros((FP, 128), np.float32)
    WA[0:943, 0:64] = U_u; WA[0:943, 64:128] = U_i
    WB = np.zeros((FP, 128), np.float32)
    WB[943:2625, 0:64] = T_i
    WB[943:2625, 64:128] = T_u; WB[2649:2668, 64:128] = M_u
    WD = np.zeros((FP, 128), np.float32)
    WD[2626:2627, 0:64] += A_u; WD[2626:2628, 0:64] += G_u
    WD[2628:2649, 0:64] += O_u
    WD[2626:2627, 64:128] += A_i; WD[2626:2628, 64:128] += G_i
    WD[2628:2649, 64:128] += O_i
    WE = np.zeros((FP, 128), np.float32)
    WE[2649:2668, 0:64] = M_i
    WE[2626:2627, 64:128] += A_u; WE[2626:2628, 64:128] += G_u
    WF_ = np.zeros((FP, 128), np.float32)
    # low half stays zero (its product partner contributes nothing);
    # the -au*au correction rides the linear chain via a derived fv row
    WF_[2626:2627, 64:128] += A_u; WF_[2628:2649, 64:128] += O_u

    w_pack = np.zeros((128, WF), np.float32)
    for t in A_TILES:
        w_pack[:, AOFF[t]:AOFF[t] + 128] = WA[t * 128:(t + 1) * 128]
    for t in B_TILES:
        w_pack[:, BOFF[t]:BOFF[t] + 128] = WB[t * 128:(t + 1) * 128]
    w_pack[:, DOFF:DOFF + 128] = WD[20 * 128:21 * 128]
    w_pack[:, EOFF:EOFF + 128] = WE[20 * 128:21 * 128]
    w_pack[:, FOFF:FOFF + 128] = WF_[20 * 128:21 * 128]
    return w_pack


def _trace_kernel(ctx: ExitStack, tc, out_d, fvt_d, w_d, lin_d, lb_d,
                  ones_d, repeat=1, loop=False):
    import concourse.mybir as mybir

    nc = tc.nc
    f32 = mybir.dt.float32
    f16 = mybir.dt.float16
    MUL = mybir.AluOpType.mult
    ADD = mybir.AluOpType.add
    COPY = mybir.ActivationFunctionType.Copy

    wpool = ctx.enter_context(tc.tile_pool(name="wpool", bufs=1))
    w_sb = wpool.tile([128, WF], f16, name="w_sb")
    # Load weights hottest-first: tile-0 A block, then the rest.
    nc.sync.dma_start(w_sb[:, 0:128], w_d[:, 0:128])
    nc.sync.dma_start(w_sb[:, 128:WF], w_d[:, 128:WF])
    lin_sb = wpool.tile([128, NKT], f32, name="lin_sb")
    nc.sync.dma_start(lin_sb[:], lin_d[:])
    lin16_sb = wpool.tile([128, NKT], f16, name="lin16_sb")
    nc.gpsimd.dma_start(lin16_sb[:], lin_d[:])  # casting DMA (f32 -> f16)
    lb_sb = wpool.tile([1, 1], f32, name="lb_sb")
    nc.sync.dma_start(lb_sb[:], lb_d[:])
    ones_sb = wpool.tile([128, 1], f16, name="ones_sb")
    nc.sync.dma_start(ones_sb[:], ones_d[:])

    fpool = ctx.enter_context(tc.tile_pool(
        name="fpool", bufs=int(os.environ.get("FFM_FBUFS", "3"))))
    pspool = ctx.enter_context(tc.tile_pool(name="pspool", bufs=1, space="PSUM"))
    spool = ctx.enter_context(tc.tile_pool(
        name="spool", bufs=int(os.environ.get("FFM_SBUFS", "2"))))
    tpool = ctx.enter_context(tc.tile_pool(name="tpool", bufs=4))
    opool = ctx.enter_context(tc.tile_pool(name="opool", bufs=2))
    out_eng = {"sync": nc.sync, "scalar": nc.scalar,
               "gpsimd": nc.gpsimd}[os.environ.get("FFM_OUTDMA", "sync")]
    lin_stt = os.environ.get("FFM_LIN_STT", "1") == "1"

    HALF = 11 * SUPER  # ring-split point of the packed super row

    folds = os.environ.get("FFM_FOLDS", "0") == "1"

    def _products(p):
        """Cross products + folds for a sub whose drains landed a sub ago
        (emitted after the NEXT sub's lin chain so the in-order DVE queue
        never stalls waiting on the ACT drains)."""
        sid = p["sid"]
        dA, dB, dD, dE, dF = p["dA"], p["dB"], p["dD"], p["dE"], p["dF"]
        st1 = spool.tile([128, NSUB], f16, tag="st1", bufs=3,
                         name=f"st1_{sid}")
        st2 = spool.tile([128, NSUB], f16, tag="st2", bufs=3,
                         name=f"st2_{sid}")
        st3 = spool.tile([128, NSUB], f16, tag="st3", bufs=3,
                         name=f"st3_{sid}")
        nc.vector.tensor_mul(st1[64:128], dA[64:128], dB[64:128])  # ui*MT
        nc.vector.tensor_mul(st1[0:64], dA[0:64], dD[0:64])        # uu*R
        nc.vector.tensor_mul(st2[64:128], dD[64:128], dB[64:128])  # S3*MT
        nc.vector.tensor_mul(st2[0:64], dE[0:64], dB[0:64])        # mi*TI
        nc.vector.tensor_mul(st3[:], dE[:], dF[:])  # [mi*0 | AG*AO]
        acc0, acc1 = p["acc0"], p["acc1"]
        if folds:
            # folds on DVE to trade PE reduce matmuls for DVE adds
            nc.vector.tensor_add(st2[:], st2[:], st3[:])
            nc.vector.tensor_add(acc0[:], acc0[:], acc1[:])
            p["red"] = (st1, st2, acc0)
        else:
            p["red"] = (st1, st2, st3, acc0, acc1)

    def _finish(p):
        """Reduce + sigmoid + store for a sub whose products are complete
        (two subs behind the matmul front, so the in-order PE queue never
        stalls on the DVE/ACT tail)."""
        red, col, logit = p["red"], p["col"], p["logit"]
        for j, srct in enumerate(red):
            nc.tensor.matmul(logit[:], ones_sb[:], srct[:],
                             start=(j == 0 and not LIN_PE_TILES),
                             stop=(j == len(red) - 1))
        out_sb = opool.tile([1, NSUB], f32, tag="out", name=f"out_{col}")
        nc.scalar.activation(out_sb[:], logit[:],
                             mybir.ActivationFunctionType.Sigmoid,
                             bias=lb_sb[0:1, 0:1], scale=1.0)
        out_eng.dma_start(out_d[0:1, col:col + NSUB], out_sb[:])

    def _body(rep, passes=1):
        pend = []  # software-pipeline stages: [-1]=needs products, [0]=needs finish
        for sp in range(passes * NSUPER):
            s = sp % NSUPER
            big = fpool.tile([128, SW], f16, tag="fv", name=f"fv_{rep}_{sp}")
            nc.sync.dma_start(big[:, 0:HALF], fvt_d[s][:, 0:HALF])
            nc.scalar.dma_start(big[:, HALF:SW], fvt_d[s][:, HALF:SW])
            for sub in range(SUPER // NSUB):
                def rhs(t):
                    o = t * SUPER + sub * NSUB
                    return big[:, o:o + NSUB]
                sid = f"{rep}_{sp}_{sub}"
                psA = pspool.tile([128, NSUB], f32, tag="psA",
                                  name=f"psA_{sid}")
                psB = pspool.tile([128, NSUB], f32, tag="psB", bufs=2,
                                  name=f"psB_{sid}")
                logit = pspool.tile([1, NSUB], f32, tag="logit", bufs=2,
                                    name=f"logit_{sid}")
                psD = pspool.tile([128, NSUB], f32, tag="psD", name=f"psD_{sid}")
                psE = pspool.tile([128, NSUB], f32, tag="psE", name=f"psE_{sid}")
                psF = pspool.tile([128, NSUB], f32, tag="psF", name=f"psF_{sid}")
                dA = spool.tile([128, NSUB], f16, tag="dA", name=f"dA_{sid}")
                accs = [None, None]   # DVE parities (hide RAW latency)
                ndve = 0
                npe = 0
                for t in range(NKT):
                    if t in A_TILES:
                        nc.tensor.matmul(psA[:], w_sb[:, AOFF[t]:AOFF[t] + 128],
                                         rhs(t), start=(t == 0), stop=(t == 7))
                    if t in B_TILES:
                        nc.tensor.matmul(psB[:], w_sb[:, BOFF[t]:BOFF[t] + 128],
                                         rhs(t), start=(t == 7), stop=(t == 20))
                    if t == 20:
                        nc.tensor.matmul(psD[:], w_sb[:, DOFF:DOFF + 128],
                                         rhs(t), start=True, stop=True)
                        nc.tensor.matmul(psE[:], w_sb[:, EOFF:EOFF + 128],
                                         rhs(t), start=True, stop=True)
                        nc.tensor.matmul(psF[:], w_sb[:, FOFF:FOFF + 128],
                                         rhs(t), start=True, stop=True)
                    # linear term: per-partition-scalar multiply-accumulate,
                    # split across PE (M=1 matmuls into the logit PSUM,
                    # emitted as one group after the blocks) and DVE
                    wcol = lin_sb[:, t:t + 1]
                    if t in LIN_PE_TILES:
                        pass  # emitted below, grouped with the reduces
                    else:
                        par = ndve % 2
                        ndve += 1
                        if accs[par] is None:
                            at = spool.tile([128, NSUB], f16, tag=f"acc{par}",
                                            bufs=3, name=f"acc{par}_{sid}")
                            nc.vector.tensor_single_scalar(at[:], rhs(t),
                                                           wcol, MUL)
                            accs[par] = at
                        elif lin_stt:
                            nc.vector.scalar_tensor_tensor(
                                accs[par][:], rhs(t), wcol, accs[par][:],
                                MUL, ADD)
                        else:
                            tmp = tpool.tile([128, NSUB], f16, tag="tmp",
                                             name=f"tmp_{sid}_{t}")
                            nc.vector.tensor_single_scalar(tmp[:], rhs(t),
                                                           wcol, MUL)
                            nc.vector.tensor_add(accs[par][:], accs[par][:],
                                                 tmp[:])
                    if t == 7:
                        # drain A mid-loop, right after its chain stops
                        nc.scalar.copy(dA[:], psA[:])
                # linear-term M=1 matmuls, grouped so the PE switches from
                # 128-row to 1-row output tiles only once per sub (an
                # interleaved switch costs ~340ns of pipeline drain each)
                for j, t in enumerate(LIN_PE_TILES):
                    nc.tensor.matmul(logit[:], lin16_sb[:, t:t + 1],
                                     rhs(t), start=(j == 0), stop=False)
                # drains: PSUM -> fp16 SBUF on the ACT engine (ready as soon
                # as this sub's t20 matmuls land, during the next lin chain)
                dB = spool.tile([128, NSUB], f16, tag="dB", name=f"dB_{sid}")
                nc.scalar.copy(dB[:], psB[:])
                dD = spool.tile([128, NSUB], f16, tag="dD", name=f"dD_{sid}")
                nc.scalar.copy(dD[:], psD[:])
                dE = spool.tile([128, NSUB], f16, tag="dE", name=f"dE_{sid}")
                nc.scalar.copy(dE[:], psE[:])
                dF = spool.tile([128, NSUB], f16, tag="dF", name=f"dF_{sid}")
                nc.scalar.copy(dF[:], psF[:])
                # retire older subs: products one sub behind, reduce two
                # (depth 1 = retire previous sub fully right here)
                depth = int(os.environ.get("FFM_DEPTH", "1"))
                if pend:
                    _products(pend[-1])
                while len(pend) >= depth:
                    _finish(pend.pop(0))
                pend.append({"sid": sid, "dA": dA, "dB": dB, "dD": dD,
                             "dE": dE, "dF": dF,
                             "acc0": accs[0], "acc1": accs[1],
                             "logit": logit,
                             "col": s * SUPER + sub * NSUB})
        _products(pend[-1])
        for p in pend:
            _finish(p)

    if loop and repeat > 1:
        # benchmarking mode: run the identical body `repeat` times inside one
        # NEFF via a hardware loop. Two full passes per iteration with the
        # software pipeline flowing across them halves the drain-bubble the
        # in-order engines pay at each loop-boundary.
        if repeat % 16 == 0:
            with tc.For_i(0, repeat // 16, 1):
                _body(0, passes=16)
        elif repeat % 8 == 0:
            with tc.For_i(0, repeat // 8, 1):
                _body(0, passes=8)
        elif repeat % 4 == 0:
            with tc.For_i(0, repeat // 4, 1):
                _body(0, passes=4)
        elif repeat % 2 == 0:
            with tc.For_i(0, repeat // 2, 1):
                _body(0, passes=2)
        else:
            with tc.For_i(0, repeat, 1):
                _body(0)
    else:
        for rep in range(repeat):
            _body(rep)


_MODULES = {}


def get_module(repeat=1, loop=False):
    """Build (once per config) and return the compiled Bass module."""
    key = (repeat, loop)
    if key in _MODULES:
        return _MODULES[key]

    import concourse.bacc as bacc
    import concourse.tile as tile
    import concourse.mybir as mybir

    nc = bacc.Bacc("TRN2", debug=False, enable_asserts=False,
                   num_devices=NCORES)
    fvt_d = nc.dram_tensor("fvt", (NSUPER, 128, SW), mybir.dt.float16,
                           kind="ExternalInput").ap()
    w_d = nc.dram_tensor("wpack", (128, WF), mybir.dt.float16,
                         kind="ExternalInput").ap()
    lin_d = nc.dram_tensor("lin32", (128, NKT), mybir.dt.float32,
                           kind="ExternalInput").ap()
    lb_d = nc.dram_tensor("linb", (1, 1), mybir.dt.float32,
                          kind="ExternalInput").ap()
    ones_d = nc.dram_tensor("ones16", (128, 1), mybir.dt.float16,
                            kind="ExternalInput").ap()
    out_d = nc.dram_tensor("out", (1, BL), mybir.dt.float32,
                           kind="ExternalOutput").ap()

    with tile.TileContext(nc) as tc, ExitStack() as ctx:
        _trace_kernel(ctx, tc, out_d, fvt_d, w_d, lin_d, lb_d,
                      ones_d, repeat=repeat, loop=loop)
    nc.compile()
    _MODULES[key] = nc
    return nc


def prepare_in_maps(inputs):
    """Host-side sharding: batch-split fv, pack each shard super-major as
    [NSUPER, 128, NKT*SUPER] fp16 (one contiguous DMA per super-chunk),
    replicate the packed weights."""
    fv = np.ascontiguousarray(np.asarray(inputs["feature_vector"], np.float32))
    assert fv.shape == (B, F)
    tables = {k: np.asarray(v, np.float32) for k, v in inputs.items()
              if k != "feature_vector"}
    w_pack = np.ascontiguousarray(_build_w_pack(tables), np.float16)
    lw = np.zeros(FP, np.float32)
    lw[:F] = tables["lin_w"][0]
    # -au*au correction rides the linear chain: a derived fv row holds
    # fv_2626^2 and its linear weight is -||A_u||^2
    lw[F + 1] = -float((tables["age_user_w"][0] ** 2).sum())
    lin32 = np.ascontiguousarray(lw.reshape(NKT, 128).T)
    lb = tables["lin_b"].reshape(1, 1)
    ones16 = np.ones((128, 1), np.float16)

    in_maps = []
    for c in range(NCORES):
        fvt = np.zeros((FP, BL), np.float16)
        fvt[:F] = fv[c * BL:(c + 1) * BL].T
        fvt[F + 1] = fv[c * BL:(c + 1) * BL, 2626] ** 2
        # [t*128+p, s*1024+c] -> [s, p, t*1024+c]
        fvt = np.ascontiguousarray(
            fvt.reshape(NKT, 128, NSUPER, SUPER).transpose(2, 1, 0, 3)
               .reshape(NSUPER, 128, SW))
        in_maps.append({"fvt": fvt, "wpack": w_pack, "lin32": lin32,
                        "linb": lb, "ones16": ones16})
    return in_maps


def kernel(**inputs) -> np.ndarray:
    # Tracing needs the axon NTFF hook, which this environment lacks; make
    # sure a stray BASS_TRACE=1 can't crash the run.
    os.environ["BASS_NEVER_TRACE"] = "1"
    from concourse import bass_utils

    in_maps = prepare_in_maps(inputs)
    nc = get_module()
    try:
        res = bass_utils.run_bass_kernel_spmd(nc, in_maps,
                                              core_ids=list(range(NCORES)))
    except Exception:
        # transient NRT device errors have been observed on this fabric;
        # one retry after a short pause usually succeeds
        import time
        time.sleep(15)
        res = bass_utils.run_bass_kernel_spmd(nc, in_maps,
                                              core_ids=list(range(NCORES)))
    out = np.concatenate([r["out"].reshape(BL) for r in res.results])
    return out.reshape(B, 1).astype(np.float32)

